# revision 1
# baseline (speedup 1.0000x reference)
"""HGT (heterogeneous graph transformer) kernel for 8 Trainium2 NeuronCores.

Strategy (row-sharded hybrid):
  - Node rows (users 20000, items 50000) are partitioned across the 8 cores.
  - The dense, shape-static phases run on the NeuronCores as Bass/Tile SPMD
    kernels (row-sharded, weights replicated):
      stage-1 (per layer, per node type): Y = x @ Wbig, where Wbig folds the
        kqv projection together with the relation transforms A_k (scaled by
        p_rel/sqrt(D)) and A_v as 128x128 block-diagonal factors, so the
        per-edge einsums of the reference collapse into pure gathers;
      final: fin = x_final @ W_lin (the shared output linear).
  - The data-dependent edge phase (gather by src, segment softmax over dst,
    scatter-add) and the small gelu/skip pointwise update run host-side
    between launches, acting as the gather/unshard step of the sharding hint.

Each device program is compiled once per process and reused across launches;
all 8 cores run the same NEFF on different row shards (SPMD).

Implementation notes (hard-won):
  - walrus codegen allows only ONE sync-wait on Matmult S3_LW; funneling all
    PE deps through a single DVE semaphore (staging copies) plus bacc.Bacc
    compile() keeps every instruction within the wait budget.
  - Matmuls with <128 output partitions crash the device (NRT 101); pad the
    weight free dim to 128.
"""

import os
import sys
import numpy as np

sys.path.insert(0, "/opt/trn_rl_repo")

H, D = 8, 16
HID = H * D
NU, NI = 20000, 50000
L = 2
N_CORES = 8
P = 128

_SQRT1_2 = np.float32(1.0 / np.sqrt(2.0))


def _gelu(x):
    from scipy.special import erf
    x64 = x.astype(np.float64)
    return (0.5 * x64 * (1.0 + erf(x64 * (1.0 / np.sqrt(2.0))))).astype(np.float32)


def _sigmoid(x):
    return 1.0 / (1.0 + np.exp(-np.float64(x)))


def _segment_softmax_agg(scores, vals, dst, n):
    """scores [E,H], vals [E,H,D], dst [E] -> [n, H, D]  (matches reference)."""
    E = scores.shape[0]
    m = np.full((n, H), -np.inf, dtype=np.float32)
    np.maximum.at(m, dst, scores)
    m_fin = np.where(np.isfinite(m), m, 0.0).astype(np.float32)
    e = np.exp(scores - m_fin[dst])
    den = np.zeros((n, H), dtype=np.float32)
    np.add.at(den, dst, e)
    den = den + np.float32(1e-16)
    alpha = e / den[dst]
    out = np.zeros((n, H, D), dtype=np.float32)
    np.add.at(out, dst, vals * alpha[..., None])
    return out


# ----------------------------------------------------------------------------
# Bass device kernel: row-sharded dense matmul stack.
# One program, reused for all launches. Computes, per core, for a row shard:
#   out = act(xT.T @ W)  with act in {relu, gelu, none}
# packed as a fixed sequence of (tag, rows, K, N, act) stages whose operands
# arrive in DRAM inputs. Stages are compiled statically below.
# ----------------------------------------------------------------------------

_COLS_T = 512            # rows processed per matmul (4 x 128, one PSUM bank)
_ROWS_SH = 9216          # padded rows per core shard (70000/8 = 8750 -> 18 tiles)
_NT = _ROWS_SH // _COLS_T  # 18 tiles


def _build_mm_program():
    """Program: finT[0:64, r] = (W.T @ x.T)  per 512-row slab, fp32.

    lhsT = W [128,64] loads once into the PE array; rhs streams x.T slabs.
    Output is produced transposed ([64, rows]); host transposes back.
    """
    import concourse.bacc as bacc
    import concourse.mybir as mybir
    import concourse.tile as tile

    nc = bacc.Bacc("TRN2", target_bir_lowering=False, debug=False)
    xT = nc.dram_tensor("xT", [P, _ROWS_SH], mybir.dt.float32, kind="ExternalInput")
    W = nc.dram_tensor("W", [P, P], mybir.dt.float32, kind="ExternalInput")
    finT = nc.dram_tensor("finT", [P, _ROWS_SH], mybir.dt.float32,
                          kind="ExternalOutput")

    with tile.TileContext(nc) as tc:
        with (
            tc.tile_pool(name="w", bufs=1) as wp,
            tc.tile_pool(name="x", bufs=4) as xp,
            tc.tile_pool(name="o", bufs=4) as op,
            tc.tile_pool(name="ps", bufs=2, space="PSUM") as pp,
        ):
            # All PE dependencies are funneled through the DVE semaphore so
            # each Matmult needs exactly ONE sync wait (walrus S3_LW limit).
            w_raw = wp.tile([P, P], mybir.dt.float32, tag="wr")
            nc.sync.dma_start(out=w_raw[:], in_=W[:, :])
            w_sb = wp.tile([P, P], mybir.dt.float32)
            nc.vector.tensor_copy(out=w_sb[:], in_=w_raw[:])
            for t in range(_NT):
                sl = slice(t * _COLS_T, (t + 1) * _COLS_T)
                x_sb = xp.tile([P, _COLS_T], mybir.dt.float32)
                nc.sync.dma_start(out=x_sb[:], in_=xT[:, sl])
                x2_sb = xp.tile([P, _COLS_T], mybir.dt.float32, tag="x2")
                nc.vector.tensor_copy(out=x2_sb[:], in_=x_sb[:])
                ps = pp.tile([P, _COLS_T], mybir.dt.float32, space="PSUM")
                nc.tensor.matmul(out=ps[:], lhsT=w_sb[:], rhs=x2_sb[:],
                                 start=True, stop=True)
                o_sb = op.tile([P, _COLS_T], mybir.dt.float32)
                nc.vector.tensor_copy(out=o_sb[:], in_=ps[:])
                nc.sync.dma_start(out=finT[:, sl], in_=o_sb[:])
    nc.compile()
    return nc


_S1_ROWS = 6400          # padded rows per core shard for stage-1 (50000/8 -> 6250)
_S1_C = 640              # output channels: kt_a|kt_b|vt_a|vt_b|q


def _build_stage1_program():
    """Y[rows, 640] = x[rows,:] @ Wbig[128, 640], fp32, per 128-row tile.

    Computes the folded projections (k@A_k blockdiag, v@A_v blockdiag, q)
    for one node-type shard. Split into 512+128 matmuls (PSUM bank limit).
    """
    import concourse.bacc as bacc
    import concourse.mybir as mybir
    import concourse.tile as tile

    nc = bacc.Bacc("TRN2", target_bir_lowering=False, debug=False)
    xT = nc.dram_tensor("xT", [P, _S1_ROWS], mybir.dt.float32, kind="ExternalInput")
    W = nc.dram_tensor("W", [P, _S1_C], mybir.dt.float32, kind="ExternalInput")
    Y = nc.dram_tensor("Y", [_S1_ROWS, _S1_C], mybir.dt.float32, kind="ExternalOutput")

    nt = _S1_ROWS // P
    with tile.TileContext(nc) as tc:
        with (
            tc.tile_pool(name="w", bufs=1) as wp,
            tc.tile_pool(name="x", bufs=4) as xp,
            tc.tile_pool(name="o", bufs=4) as op,
            tc.tile_pool(name="ps", bufs=2, space="PSUM") as pp,
        ):
            w_raw = wp.tile([P, _S1_C], mybir.dt.float32, tag="wr")
            nc.sync.dma_start(out=w_raw[:], in_=W[:, :])
            w_sb = wp.tile([P, _S1_C], mybir.dt.float32)
            nc.vector.tensor_copy(out=w_sb[:], in_=w_raw[:])
            for t in range(nt):
                rs = slice(t * P, (t + 1) * P)
                x_sb = xp.tile([P, P], mybir.dt.float32)
                nc.sync.dma_start(out=x_sb[:], in_=xT[:, rs])
                x2_sb = xp.tile([P, P], mybir.dt.float32, tag="x2")
                nc.vector.tensor_copy(out=x2_sb[:], in_=x_sb[:])
                ps1 = pp.tile([P, 512], mybir.dt.float32, space="PSUM")
                nc.tensor.matmul(out=ps1[:], lhsT=x2_sb[:], rhs=w_sb[:, 0:512],
                                 start=True, stop=True)
                ps2 = pp.tile([P, 128], mybir.dt.float32, space="PSUM", tag="ps2")
                nc.tensor.matmul(out=ps2[:], lhsT=x2_sb[:], rhs=w_sb[:, 512:640],
                                 start=True, stop=True)
                o_sb = op.tile([P, _S1_C], mybir.dt.float32)
                nc.vector.tensor_copy(out=o_sb[:, 0:512], in_=ps1[:])
                nc.vector.tensor_copy(out=o_sb[:, 512:640], in_=ps2[:])
                nc.sync.dma_start(out=Y[rs, :], in_=o_sb[:])
    nc.compile()
    return nc


_prog_cache = {}
_LAST_HW_NS = None
_HW_NS_TOTAL = 0


def _launch(nc, in_maps):
    import time
    from concourse import bass_utils
    global _LAST_HW_NS, _HW_NS_TOTAL
    t0 = time.time()
    res = bass_utils.run_bass_kernel_spmd(
        nc, in_maps, core_ids=list(range(N_CORES)))
    dt_ns = int((time.time() - t0) * 1e9)
    if res.exec_time_ns:
        dt_ns = int(res.exec_time_ns)
    _LAST_HW_NS = dt_ns
    _HW_NS_TOTAL += dt_ns
    return res


def _run_stage1(x, Wbig):
    """x [N,128] @ Wbig [128,640] on 8 cores, row-sharded; returns [N,640]."""
    if "s1" not in _prog_cache:
        _prog_cache["s1"] = _build_stage1_program()
    nc = _prog_cache["s1"]

    n_tot = x.shape[0]
    rows_pc = (n_tot + N_CORES - 1) // N_CORES
    Wb = np.ascontiguousarray(Wbig.astype(np.float32))
    in_maps = []
    for c in range(N_CORES):
        sh = x[c * rows_pc:(c + 1) * rows_pc]
        pad = np.zeros((_S1_ROWS, P), dtype=np.float32)
        pad[: sh.shape[0]] = sh
        in_maps.append({"xT": np.ascontiguousarray(pad.T), "W": Wb})
    res = _launch(nc, in_maps)
    outs = [res.results[c]["Y"][:rows_pc] for c in range(N_CORES)]
    return np.concatenate(outs, axis=0)[:n_tot]


def _blockdiag(blocks):
    """blocks [H,D,D] -> [HID, HID] block-diagonal."""
    out = np.zeros((HID, HID), dtype=np.float32)
    for h in range(H):
        out[h * D:(h + 1) * D, h * D:(h + 1) * D] = blocks[h]
    return out


def _run_final_linear(xcat, W_lin):
    """xcat [70000,128] @ W_lin [128,64] on 8 cores, row-sharded."""
    if "mm" not in _prog_cache:
        _prog_cache["mm"] = _build_mm_program()
    nc = _prog_cache["mm"]

    n_tot = xcat.shape[0]
    rows_pc = (n_tot + N_CORES - 1) // N_CORES  # 8750
    in_maps = []
    for c in range(N_CORES):
        sh = xcat[c * rows_pc:(c + 1) * rows_pc]
        pad = np.zeros((_ROWS_SH, P), dtype=np.float32)
        pad[: sh.shape[0]] = sh
        Wp = np.zeros((P, P), dtype=np.float32)
        Wp[:, :64] = W_lin.astype(np.float32)
        in_maps.append({
            "xT": np.ascontiguousarray(pad.T),
            "W": Wp,
        })
    res = _launch(nc, in_maps)
    outs = [res.results[c]["finT"][:64].T[:rows_pc] for c in range(N_CORES)]
    return np.concatenate(outs, axis=0)[:n_tot]


def kernel(**inp):
    x_user = np.asarray(inp["x_user"], dtype=np.float32)
    x_item = np.asarray(inp["x_item"], dtype=np.float32)
    A_k = np.asarray(inp["A_k"], dtype=np.float32)
    A_v = np.asarray(inp["A_v"], dtype=np.float32)
    p_rel = np.asarray(inp["p_rel"], dtype=np.float32)
    inv_sqrt_d = np.float32(1.0 / np.sqrt(np.float32(D)))

    edges = {
        0: (np.asarray(inp["edge_src_ui"]), np.asarray(inp["edge_dst_ui"])),
        1: (np.asarray(inp["edge_src_iu"]), np.asarray(inp["edge_dst_iu"])),
        2: (np.asarray(inp["edge_src_uu"]), np.asarray(inp["edge_dst_uu"])),
    }

    xu = np.maximum(x_user @ inp["W_in_user"] + inp["b_in_user"], 0.0).astype(np.float32)
    xi = np.maximum(x_item @ inp["W_in_item"] + inp["b_in_item"], 0.0).astype(np.float32)

    for l in range(L):
        # Fold relation transforms A_k (with p_rel/sqrt(D) scale) and A_v into
        # the kqv projection weights -> one big on-device matmul per type.
        Wk_u, Wq_u, Wv_u = np.split(np.asarray(inp["W_kqv_user"][l], np.float32), 3, axis=1)
        bk_u, bq_u, bv_u = np.split(np.asarray(inp["b_kqv_user"][l], np.float32), 3)
        Wk_i, Wq_i, Wv_i = np.split(np.asarray(inp["W_kqv_item"][l], np.float32), 3, axis=1)
        bk_i, bq_i, bv_i = np.split(np.asarray(inp["b_kqv_item"][l], np.float32), 3)

        def bk_sc(r):
            return _blockdiag(A_k[l, r] * (p_rel[l, r] * inv_sqrt_d)[:, None, None])

        Bk0, Bk1, Bk2 = bk_sc(0), bk_sc(1), bk_sc(2)
        Bv0, Bv1, Bv2 = (_blockdiag(A_v[l, r]) for r in range(3))

        Wbig_u = np.concatenate(
            [Wk_u @ Bk0, Wk_u @ Bk2, Wv_u @ Bv0, Wv_u @ Bv2, Wq_u], axis=1)
        bbig_u = np.concatenate([bk_u @ Bk0, bk_u @ Bk2, bv_u @ Bv0, bv_u @ Bv2, bq_u])
        Wbig_i = np.concatenate(
            [Wk_i @ Bk1, Wv_i @ Bv1, Wq_i, np.zeros((HID, 2 * HID), np.float32)], axis=1)
        bbig_i = np.concatenate([bk_i @ Bk1, bv_i @ Bv1, bq_i, np.zeros(2 * HID, np.float32)])

        Yu = _run_stage1(xu, Wbig_u) + bbig_u[None, :]
        Yi = _run_stage1(xi, Wbig_i) + bbig_i[None, :]
        kt0 = Yu[:, 0:128].reshape(NU, H, D)
        kt2 = Yu[:, 128:256].reshape(NU, H, D)
        vt0 = Yu[:, 256:384].reshape(NU, H, D)
        vt2 = Yu[:, 384:512].reshape(NU, H, D)
        q_u = Yu[:, 512:640].reshape(NU, H, D)
        kt1 = Yi[:, 0:128].reshape(NI, H, D)
        vt1 = Yi[:, 128:256].reshape(NI, H, D)
        q_i = Yi[:, 256:384].reshape(NI, H, D)

        def edge_sc(kt_t, vt_t, q_dst, src, dst):
            sc = (q_dst[dst] * kt_t[src]).sum(-1).astype(np.float32)
            return sc, vt_t[src]

        s_ui, m_ui = edge_sc(kt0, vt0, q_i, *edges[0])
        s_iu, m_iu = edge_sc(kt1, vt1, q_u, *edges[1])
        s_uu, m_uu = edge_sc(kt2, vt2, q_u, *edges[2])

        out_i = _segment_softmax_agg(s_ui, m_ui, edges[0][1], NI).reshape(NI, HID)
        out_u = _segment_softmax_agg(
            np.concatenate([s_iu, s_uu]),
            np.concatenate([m_iu, m_uu]),
            np.concatenate([edges[1][1], edges[2][1]]), NU).reshape(NU, HID)

        a_u = (_gelu(out_u) @ inp["W_out_user"][l] + inp["b_out_user"][l]).astype(np.float32)
        a_i = (_gelu(out_i) @ inp["W_out_item"][l] + inp["b_out_item"][l]).astype(np.float32)
        g_u = np.float32(_sigmoid(inp["skip_user"][l]))
        g_i = np.float32(_sigmoid(inp["skip_item"][l]))
        xu = np.maximum(g_u * a_u + (1.0 - g_u) * xu, 0.0).astype(np.float32)
        xi = np.maximum(g_i * a_i + (1.0 - g_i) * xi, 0.0).astype(np.float32)

    xcat = np.concatenate([xu, xi], axis=0).astype(np.float32)
    out = _run_final_linear(xcat, np.asarray(inp["W_lin"], dtype=np.float32))
    out = out + np.asarray(inp["b_lin"], dtype=np.float32)[None, :]
    return out.astype(np.float32)



# revision 10
# speedup vs baseline: 4.6445x; 4.6445x over previous
"""HGT (heterogeneous graph transformer) on 8 Trainium2 NeuronCores.

Single-launch design (dst-sharded, per the edge-parallel sharding hint):

  Host (cheap integer prep):
    - Destination nodes of each type are dealt round-robin across the 8
      cores in descending total-in-degree order.  Each core's local node
      order is therefore degree-sorted, and all cores share one static
      per-tile max-degree schedule (exact, computed from the data and baked
      into the compiled program).
    - All per-edge gather indices are pre-translated into permuted table
      rows; per-(node,group) slot pads point at an all-zero table row and
      are corrected by a per-node pad-count subtraction on the device.
    - Item-table rows can exceed int16 (50000 rows + padding), so item-src
      edges are split into two gather groups (A/B) at a core-aligned river.

  Device (one Bass program, one launch, SPMD on 8 cores):
    - input projection (relu(x @ W_in + b)) into a resident SBUF transposed
      activation table xT per node type (feature-major, matmul-ready).
    - per layer:
        stage-1: local-shard matmuls produce k/v tables (relation transforms
          A_k (scaled by p_rel/sqrt(D)) and A_v folded in as block-diagonal
          factors) written to DRAM; q stays resident in SBUF.
        AllGather of the k|v tables across the 8 cores (the only collective).
        edge phase: per 128-node tile, per relation group, dma_gather of
          fused 256-float k|v slot rows, segment softmax entirely in SBUF
          (broadcast multiply + strided reduces; exp without max-subtraction
          - scores are O(5)), then gelu -> W_out matmul with bias and
          sigmoid-gated skip folded in as extra PSUM-accumulated matmuls.
    - final shared linear from the resident xT tiles; host undoes the node
      permutation on the gathered per-core outputs.

  Everything per-core-varying arrives as input data; the program itself is
  identical across cores (no partition-id branches).
"""

import sys

import numpy as np

sys.path.insert(0, "/opt/trn_rl_repo")

H = 8
DH = 16
HID = 128
L = 2
P = 128
DCAP = 8             # max slots per gather unit (dma_gather caps at 1024 idxs)
IDX_I16_LIM = 32768


def _ceil(a, b):
    return (a + b - 1) // b


class Cfg:
    def __init__(self, NU, NI, ncores):
        self.NU, self.NI, self.C = NU, NI, ncores
        assert NU % ncores == 0 and NI % ncores == 0
        self.upc = NU // ncores
        self.ipc = NI // ncores
        # always leave at least one pad row (the gather pad slots' zero row)
        self.upad = _ceil(self.upc + 1, P) * P
        self.ipad = _ceil(self.ipc + 1, P) * P
        self.UT = self.upad // P
        self.IT = self.ipad // P
        # core-aligned int16 river for the item table
        self.bsplit = (IDX_I16_LIM // self.ipad) * self.ipad
        if self.bsplit >= self.C * self.ipad:
            self.bsplit = self.C * self.ipad  # no B range needed
        # schedules, filled by _prep_graph:
        self.DA = self.DB = self.DU = self.DI = None


# ---------------------------------------------------------------------------
# host-side graph preprocessing
# ---------------------------------------------------------------------------

def _assign(deg_total, n, C):
    order = np.argsort(-deg_total, kind="stable")
    core = np.empty(n, np.int32)
    pos = np.empty(n, np.int32)
    ar = np.arange(n, dtype=np.int32)
    core[order] = ar % C
    pos[order] = ar // C
    return core, pos


def _tile_sched(deg, pos, ntiles):
    Ds = np.zeros(ntiles, np.int64)
    np.maximum.at(Ds, pos // P, deg.astype(np.int64))
    return Ds.astype(np.int32)


def _slot_fill(idx_arr, src_local, dst, core, pos, base):
    """Place each edge's src table row into its (core, token) slot.

    idx_arr: [C, TOK] int16 (prefilled with the group's zero-row).
    base: [ntiles] token base of this group's block per tile.
    """
    o = np.argsort(dst, kind="stable")
    ds = dst[o]
    ss = src_local[o]
    first = np.r_[True, ds[1:] != ds[:-1]] if len(ds) else np.zeros(0, bool)
    runstart = np.flatnonzero(first)
    runid = np.cumsum(first) - 1
    dwithin = np.arange(len(ds)) - runstart[runid]
    c = core[ds]
    p = pos[ds]
    tok = base[p // P] + dwithin * P + (p % P)
    idx_arr[c, tok] = ss.astype(np.int16)


def _npads(cfg, Ds_list, deg_list, core, pos, pad, ntiles):
    """[C, P, ntiles] float32: per-lane pad-slot counts summed over groups."""
    C = cfg.C
    out = np.zeros((C, P, ntiles), np.float32)
    for Ds, deg in zip(Ds_list, deg_list):
        degmat = np.zeros((C, pad), np.int64)
        degmat[core, pos] = deg
        degmat = degmat.reshape(C, ntiles, P)
        out += (Ds[None, None, :] - degmat.transpose(0, 2, 1)).astype(np.float32)
    return out


def _wrap_idx(arr):
    """[C, TOK] -> [C, 128, TOK//16]: token i at [i % 16, i // 16], and the
    16-partition wrap replicated to all 8 groups of 16 partitions (each Q7
    sub-core of the gpsimd engine reads its own 16-partition copy)."""
    C, TOK = arr.shape
    assert TOK % 16 == 0
    w = arr.reshape(C, TOK // 16, 16).transpose(0, 2, 1)  # [C, 16, TOK//16]
    return np.ascontiguousarray(np.tile(w, (1, 8, 1)))


def _prep_graph(cfg, e_ui, e_iu, e_uu):
    NU, NI, C = cfg.NU, cfg.NI, cfg.C
    deg_iu = np.bincount(e_iu[1], minlength=NU)
    deg_uu = np.bincount(e_uu[1], minlength=NU)
    deg_ui = np.bincount(e_ui[1], minlength=NI)

    u_core, u_pos = _assign(deg_iu + deg_uu, NU, C)
    i_core, i_pos = _assign(deg_ui, NI, C)
    u_row = (u_core.astype(np.int64) * cfg.upad + u_pos)
    i_row = (i_core.astype(np.int64) * cfg.ipad + i_pos)

    # item-src split at the river
    s_iu, d_iu = e_iu
    rows_iu = i_row[s_iu]
    isB = rows_iu >= cfg.bsplit
    degA = np.bincount(d_iu[~isB], minlength=NU)
    degB = np.bincount(d_iu[isB], minlength=NU)

    cfg.DA = _tile_sched(degA, u_pos, cfg.UT)
    cfg.DB = _tile_sched(degB, u_pos, cfg.UT)
    cfg.DU = _tile_sched(deg_uu, u_pos, cfg.UT)
    cfg.DI = _tile_sched(deg_ui, i_pos, cfg.IT)

    # token bases (shared across cores)
    baseA = np.zeros(cfg.UT, np.int64)
    baseB = np.zeros(cfg.UT, np.int64)
    baseU = np.zeros(cfg.UT, np.int64)
    off = 0
    for t in range(cfg.UT):
        baseA[t] = off
        off += P * int(cfg.DA[t])
        baseB[t] = off
        off += P * int(cfg.DB[t])
        baseU[t] = off
        off += P * int(cfg.DU[t])
    cfg.TOKU = off
    baseI = np.zeros(cfg.IT, np.int64)
    off = 0
    for t in range(cfg.IT):
        baseI[t] = off
        off += P * int(cfg.DI[t])
    cfg.TOKI = off
    cfg.baseA, cfg.baseB, cfg.baseU, cfg.baseI = baseA, baseB, baseU, baseI

    uzrow = cfg.upc            # core 0's first user pad row (all-zero)
    izrowA = cfg.ipc           # core 0's first item pad row
    izrowB = cfg.ipc           # first-B-core's pad row, B-local

    idx_u = np.empty((C, max(cfg.TOKU, 16)), np.int16)
    for t in range(cfg.UT):
        idx_u[:, baseA[t]:baseA[t] + P * int(cfg.DA[t])] = izrowA
        idx_u[:, baseB[t]:baseB[t] + P * int(cfg.DB[t])] = izrowB
        idx_u[:, baseU[t]:baseU[t] + P * int(cfg.DU[t])] = uzrow
    idx_i = np.full((C, max(cfg.TOKI, 16)), uzrow, np.int16)

    assert u_row.max() < IDX_I16_LIM
    _slot_fill(idx_u, rows_iu[~isB], d_iu[~isB], u_core, u_pos, baseA)
    _slot_fill(idx_u, rows_iu[isB] - cfg.bsplit, d_iu[isB], u_core, u_pos, baseB)
    _slot_fill(idx_u, u_row[e_uu[0]], e_uu[1], u_core, u_pos, baseU)
    _slot_fill(idx_i, u_row[e_ui[0]], e_ui[1], i_core, i_pos, baseI)

    npad_u = _npads(cfg, [cfg.DA, cfg.DB, cfg.DU], [degA, degB, deg_uu],
                    u_core, u_pos, cfg.upad, cfg.UT)
    npad_i = _npads(cfg, [cfg.DI], [deg_ui], i_core, i_pos, cfg.ipad, cfg.IT)

    return dict(u_core=u_core, u_pos=u_pos, i_core=i_core, i_pos=i_pos,
                idx_u=_wrap_idx(idx_u), idx_i=_wrap_idx(idx_i),
                npad_u=npad_u, npad_i=npad_i)


# ---------------------------------------------------------------------------
# weight folding (host)
# ---------------------------------------------------------------------------

def _blockdiag(blocks):
    hb, d, _ = blocks.shape
    out = np.zeros((hb * d, hb * d), np.float32)
    for h in range(hb):
        out[h * d:(h + 1) * d, h * d:(h + 1) * d] = blocks[h]
    return out


def _sigmoid(x):
    return 1.0 / (1.0 + np.exp(-np.float64(x)))


def _fold_weights(inp):
    """Per-layer folded weights. Table column layouts:
       user k|v table (512): [kt0|vt0|kt2|vt2]  (ui-src | uu-src)
       item k|v table (256): [kt1|vt1]          (iu-src)
       q kept separate (SBUF-resident on device)."""
    A_k = np.asarray(inp["A_k"], np.float32)
    A_v = np.asarray(inp["A_v"], np.float32)
    p_rel = np.asarray(inp["p_rel"], np.float32)
    inv = np.float32(1.0 / np.sqrt(np.float32(DH)))
    W = {}
    for l in range(L):
        Wk_u, Wq_u, Wv_u = np.split(np.asarray(inp["W_kqv_user"][l], np.float32), 3, 1)
        bk_u, bq_u, bv_u = np.split(np.asarray(inp["b_kqv_user"][l], np.float32), 3)
        Wk_i, Wq_i, Wv_i = np.split(np.asarray(inp["W_kqv_item"][l], np.float32), 3, 1)
        bk_i, bq_i, bv_i = np.split(np.asarray(inp["b_kqv_item"][l], np.float32), 3)

        def bk(r):
            return _blockdiag(A_k[l, r] * (p_rel[l, r] * inv)[:, None, None])

        Bk0, Bk1, Bk2 = bk(0), bk(1), bk(2)
        Bv0, Bv1, Bv2 = (_blockdiag(A_v[l, r]) for r in range(3))

        W[("Wbigu", l)] = np.ascontiguousarray(np.concatenate(
            [Wk_u @ Bk0, Wv_u @ Bv0, Wk_u @ Bk2, Wv_u @ Bv2, Wq_u], 1))
        W[("bbigu", l)] = np.concatenate(
            [bk_u @ Bk0, bv_u @ Bv0, bk_u @ Bk2, bv_u @ Bv2, bq_u])[None, :]
        W[("Wbigi", l)] = np.ascontiguousarray(np.concatenate(
            [Wk_i @ Bk1, Wv_i @ Bv1, Wq_i], 1))
        W[("bbigi", l)] = np.concatenate([bk_i @ Bk1, bv_i @ Bv1, bq_i])[None, :]

        for ty, wkey, bkey, skey in (("u", "W_out_user", "b_out_user", "skip_user"),
                                     ("i", "W_out_item", "b_out_item", "skip_item")):
            g = np.float32(_sigmoid(np.asarray(inp[skey], np.float32)[l]))
            W[("Wout" + ty, l)] = np.ascontiguousarray(
                g * np.asarray(inp[wkey][l], np.float32))
            W[("bout" + ty, l)] = (g * np.asarray(inp[bkey][l], np.float32))[None, :]
            W[("OmgI" + ty, l)] = np.ascontiguousarray(
                (1.0 - g) * np.eye(HID, dtype=np.float32))
    W["Winu"] = np.asarray(inp["W_in_user"], np.float32)
    W["binu"] = np.asarray(inp["b_in_user"], np.float32)[None, :]
    W["Wini"] = np.asarray(inp["W_in_item"], np.float32)
    W["bini"] = np.asarray(inp["b_in_item"], np.float32)[None, :]
    W["Wlin"] = np.asarray(inp["W_lin"], np.float32)
    W["blin"] = np.asarray(inp["b_lin"], np.float32)[None, :]
    return W


# ---------------------------------------------------------------------------
# device program
# ---------------------------------------------------------------------------

def _build_program(cfg, nin_u):
    import os
    import concourse.bacc as bacc
    import concourse.mybir as mybir
    import concourse.tile as tile

    f32 = mybir.dt.float32
    i16 = mybir.dt.int16
    AX = mybir.AxisListType
    OP = mybir.AluOpType
    ACT = mybir.ActivationFunctionType

    C, upad, ipad, UT, IT = cfg.C, cfg.upad, cfg.ipad, cfg.UT, cfg.IT
    upc, ipc = cfg.upc, cfg.ipc

    phase_lim = int(os.environ.get("KPHASE", "9"))
    nc = bacc.Bacc("TRN2", target_bir_lowering=False, debug=False, num_devices=C)

    # ---- I/O ----
    xuT_in = nc.dram_tensor("xuT", [nin_u, upad], f32, kind="ExternalInput")
    xiT_in = nc.dram_tensor("xiT", [64, ipad], f32, kind="ExternalInput")
    win = {}
    for nm, shp in [("Winu", [nin_u, HID]), ("binu", [1, HID]),
                    ("Wini", [64, HID]), ("bini", [1, HID]),
                    ("Wlin", [HID, 64]), ("blin", [1, 64])]:
        win[nm] = nc.dram_tensor(nm, shp, f32, kind="ExternalInput")
    for l in range(L):
        for nm, shp in [("Wbigu", [HID, 640]), ("bbigu", [1, 640]),
                        ("Wbigi", [HID, 384]), ("bbigi", [1, 384]),
                        ("Woutu", [HID, HID]), ("boutu", [1, HID]),
                        ("OmgIu", [HID, HID]),
                        ("Wouti", [HID, HID]), ("bouti", [1, HID]),
                        ("OmgIi", [HID, HID])]:
            win[(nm, l)] = nc.dram_tensor(f"{nm}{l}", shp, f32, kind="ExternalInput")
    idxu_in = nc.dram_tensor("idxu", [P, max(cfg.TOKU, 16) // 16], i16,
                             kind="ExternalInput")
    idxi_in = nc.dram_tensor("idxi", [P, max(cfg.TOKI, 16) // 16], i16,
                             kind="ExternalInput")
    npadu_in = nc.dram_tensor("npadu", [P, UT], f32, kind="ExternalInput")
    npadi_in = nc.dram_tensor("npadi", [P, IT], f32, kind="ExternalInput")
    outu = nc.dram_tensor("outu", [upad, 64], f32, kind="ExternalOutput")
    outi = nc.dram_tensor("outi", [ipad, 64], f32, kind="ExternalOutput")

    # inline constants
    ident_d = nc.inline_tensor(np.eye(P, dtype=np.float32), "ident")
    ones_row_d = nc.inline_tensor(np.ones((1, 512), np.float32), "ones_row")
    mask_u = np.zeros((1, P), np.float32)
    mask_u[0, :upc - P * (UT - 1)] = 1.0
    mask_i = np.zeros((1, P), np.float32)
    mask_i[0, :ipc - P * (IT - 1)] = 1.0
    masku_d = nc.inline_tensor(mask_u, "masku")
    maski_d = nc.inline_tensor(mask_i, "maski")

    def ones_u(t):
        return sb["masku"] if t == UT - 1 else sb["ones"][:, 0:P]

    def ones_i(t):
        return sb["maski"] if t == IT - 1 else sb["ones"][:, 0:P]

    sb = {}

    with tile.TileContext(nc) as tc:
        with (
            tc.tile_pool(name="const", bufs=1) as cp,
            tc.tile_pool(name="resid", bufs=1) as rp,
            tc.tile_pool(name="dram", bufs=1, space="DRAM") as dp,
        ):
            # ---- constants / weights -> SBUF ----
            def load(nm, src, shp, dt=f32):
                t = cp.tile(shp, dt, tag=f"c_{nm}")
                nc.sync.dma_start(out=t[:], in_=src[:, :])
                sb[nm] = t

            load("ident", ident_d, [P, P])
            load("ones", ones_row_d, [1, 512])
            load("masku", masku_d, [1, P])
            load("maski", maski_d, [1, P])
            for nm in ("Winu", "binu", "Wini", "bini", "Wlin", "blin"):
                load(nm, win[nm], list(win[nm].shape))
            for l in range(L):
                for nm in ("Wbigu", "bbigu", "Wbigi", "bbigi", "Woutu", "boutu",
                           "OmgIu", "Wouti", "bouti", "OmgIi"):
                    load(f"{nm}{l}", win[(nm, l)], list(win[(nm, l)].shape))
            load("idxu", idxu_in, list(idxu_in.shape), i16)
            load("idxi", idxi_in, list(idxi_in.shape), i16)
            load("npadu", npadu_in, [P, UT])
            load("npadi", npadi_in, [P, IT])

            # resident activations (transposed) + q tables
            xu = rp.tile([P, upad], f32, tag="xu")
            xi = rp.tile([P, ipad], f32, tag="xi")
            qu = rp.tile([P, upad], f32, tag="qu")
            qi = rp.tile([P, ipad], f32, tag="qi")

            # DRAM k|v tables
            utab_loc = dp.tile([upad, 512], f32, tag="utl")
            itab_loc = dp.tile([ipad, 256], f32, tag="itl")
            utab = dp.tile([C * upad, 512], f32, tag="utg")
            itab = dp.tile([C * ipad, 256], f32, tag="itg")

            # ---- input projection ----
            with (
                tc.tile_pool(name="pj_in", bufs=3) as pin,
                tc.tile_pool(name="pj_ps", bufs=2, space="PSUM") as pps,
            ):
                for (xres, xdram, wnm, bnm, pad, npart) in (
                    (xu, xuT_in, "Winu", "binu", upad, nin_u),
                    (xi, xiT_in, "Wini", "bini", ipad, 64),
                ):
                    for c0 in range(0, pad, 512):
                        w = min(512, pad - c0)
                        xin = pin.tile([npart, 512], f32, tag=f"pi{npart}")
                        nc.sync.dma_start(out=xin[:, 0:w], in_=xdram[:, c0:c0 + w])
                        ps = pps.tile([P, 512], f32, tag="pj")
                        nc.tensor.matmul(out=ps[:, 0:w], lhsT=sb[wnm][:],
                                         rhs=xin[:, 0:w], start=True, stop=False)
                        nc.tensor.matmul(out=ps[:, 0:w], lhsT=sb[bnm][:],
                                         rhs=sb["ones"][:, 0:w], start=False,
                                         stop=True)
                        nc.scalar.activation(out=xres[:, c0:c0 + w], in_=ps[:, 0:w],
                                             func=ACT.Relu)
                if upad > upc:
                    nc.vector.memset(xu[:, upc:upad], 0.0)
                if ipad > ipc:
                    nc.vector.memset(xi[:, ipc:ipad], 0.0)

            for l in range(L if phase_lim >= 6 else min(L, 1)):
                if phase_lim < 2:
                    break
                # ---- stage-1: local k|v tables + resident q ----
                with (
                    tc.tile_pool(name="s1a", bufs=2, space="PSUM") as pA,
                    tc.tile_pool(name="s1b", bufs=2, space="PSUM") as pB,
                    tc.tile_pool(name="s1o", bufs=3) as pO,
                ):
                    Wb, bb = sb[f"Wbigu{l}"], sb[f"bbigu{l}"]
                    for t in range(UT):
                        lhs = xu[:, t * P:(t + 1) * P]
                        psa = pA.tile([P, 512], f32, tag="a")
                        nc.tensor.matmul(out=psa[:], lhsT=lhs, rhs=Wb[:, 0:512],
                                         start=True, stop=False)
                        nc.tensor.matmul(out=psa[:], lhsT=ones_u(t),
                                         rhs=bb[:, 0:512], start=False, stop=True)
                        psb = pB.tile([P, HID], f32, tag="b")
                        nc.tensor.matmul(out=psb[:], lhsT=lhs, rhs=Wb[:, 512:640],
                                         start=True, stop=False)
                        nc.tensor.matmul(out=psb[:], lhsT=ones_u(t),
                                         rhs=bb[:, 512:640], start=False, stop=True)
                        osb = pO.tile([P, 512], f32, tag="ou")
                        nc.vector.tensor_copy(out=osb[:], in_=psa[:])
                        nc.scalar.activation(out=qu[:, t * P:(t + 1) * P],
                                             in_=psb[:], func=ACT.Copy)
                        nc.sync.dma_start(out=utab_loc[t * P:(t + 1) * P, :],
                                          in_=osb[:])
                    Wb, bb = sb[f"Wbigi{l}"], sb[f"bbigi{l}"]
                    for t in range(IT):
                        lhs = xi[:, t * P:(t + 1) * P]
                        psa = pA.tile([P, 384], f32, tag="a")
                        nc.tensor.matmul(out=psa[:], lhsT=lhs, rhs=Wb[:],
                                         start=True, stop=False)
                        nc.tensor.matmul(out=psa[:], lhsT=ones_i(t), rhs=bb[:],
                                         start=False, stop=True)
                        osb = pO.tile([P, 256], f32, tag="oi")
                        nc.vector.tensor_copy(out=osb[:], in_=psa[:, 0:256])
                        nc.scalar.activation(out=qi[:, t * P:(t + 1) * P],
                                             in_=psa[:, 256:384], func=ACT.Copy)
                        nc.sync.dma_start(out=itab_loc[t * P:(t + 1) * P, :],
                                          in_=osb[:])

                # ---- AllGather k|v tables ----
                if phase_lim < 3:
                    continue
                groups = [list(range(C))]
                nc.gpsimd.collective_compute(
                    "AllGather", mybir.AluOpType.bypass, replica_groups=groups,
                    ins=[utab_loc[:, :].opt()], outs=[utab[:, :].opt()])
                nc.gpsimd.collective_compute(
                    "AllGather", mybir.AluOpType.bypass, replica_groups=groups,
                    ins=[itab_loc[:, :].opt()], outs=[itab[:, :].opt()])

                # ---- edge phase + node update ----
                def edge_tile(t, groups_t, q_res, npad_sb, xres, wout, bout, omgi,
                              ones_m, idx_sb):
                    den = acc.tile([P, H], f32, tag="den")
                    num = acc.tile([P, HID], f32, tag="num")
                    qv = q_res[:, t * P:(t + 1) * P].rearrange(
                        "p (h d) -> p h d", h=H)
                    first = True
                    for (in_ap, step, base, Dg) in groups_t:
                        off = 0
                        while off < Dg:
                            Dc = min(DCAP, Dg - off)
                            tok0 = int(base) + P * off
                            gb = gp.tile([P, DCAP, 256], f32, tag="gb")
                            nc.gpsimd.dma_gather(
                                gb[:, 0:Dc, :], in_ap,
                                idx_sb[:, tok0 // 16: tok0 // 16 + 8 * Dc],
                                P * Dc, P * Dc, 256, elem_step=step)
                            kview = gb[:, 0:Dc, 0:128].rearrange(
                                "p c (h d) -> p c h d", h=H)
                            vview = gb[:, 0:Dc, 128:256].rearrange(
                                "p c (h d) -> p c h d", h=H)
                            kq = tp.tile([P, DCAP, HID], f32, tag="kq")
                            kq4 = kq[:, 0:Dc, :].rearrange(
                                "p c (h d) -> p c h d", h=H)
                            nc.vector.tensor_tensor(
                                out=kq4, in0=kview,
                                in1=qv.unsqueeze(1).broadcast_to([P, Dc, H, DH]),
                                op=OP.mult)
                            s_t = sp.tile([P, DCAP, H], f32, tag="s")
                            nc.vector.tensor_reduce(
                                out=s_t[:, 0:Dc, :], in_=kq4, axis=AX.X, op=OP.add)
                            e_t = sp.tile([P, DCAP, H], f32, tag="e")
                            nc.scalar.activation(out=e_t[:, 0:Dc, :],
                                                 in_=s_t[:, 0:Dc, :], func=ACT.Exp)
                            ev = tp.tile([P, DCAP, HID], f32, tag="kq")
                            ev4 = ev[:, 0:Dc, :].rearrange(
                                "p c (h d) -> p c h d", h=H)
                            nc.vector.tensor_tensor(
                                out=ev4, in0=vview,
                                in1=e_t[:, 0:Dc, :].unsqueeze(3).broadcast_to(
                                    [P, Dc, H, DH]),
                                op=OP.mult)
                            if first:
                                nc.vector.tensor_reduce(
                                    out=den[:], in_=e_t[:, 0:Dc, :].transpose(
                                        [0, 2, 1]), axis=AX.X, op=OP.add)
                                nc.vector.tensor_reduce(
                                    out=num[:].rearrange("p (h d) -> p h d", h=H),
                                    in_=ev4.transpose([0, 2, 3, 1]),
                                    axis=AX.X, op=OP.add)
                            else:
                                dt_ = sp.tile([P, H], f32, tag="dt")
                                nc.vector.tensor_reduce(
                                    out=dt_[:], in_=e_t[:, 0:Dc, :].transpose(
                                        [0, 2, 1]), axis=AX.X, op=OP.add)
                                nc.vector.tensor_tensor(out=den[:], in0=den[:],
                                                        in1=dt_[:], op=OP.add)
                                nt_ = pp2.tile([P, HID], f32, tag="nt")
                                nc.vector.tensor_reduce(
                                    out=nt_[:].rearrange("p (h d) -> p h d", h=H),
                                    in_=ev4.transpose([0, 2, 3, 1]),
                                    axis=AX.X, op=OP.add)
                                nc.vector.tensor_tensor(out=num[:], in0=num[:],
                                                        in1=nt_[:], op=OP.add)
                            first = False
                            off += Dc
                    if first:
                        nc.vector.memset(den[:], 0.0)
                        nc.vector.memset(num[:], 0.0)
                    # segment softmax denominator (pad correction) + update
                    den1 = sp.tile([P, H], f32, tag="d1")
                    # (den - npads) can cancel to exactly 0 for fully-padded
                    # lanes (fp32 absorbs a tiny eps added first), so clamp.
                    nc.vector.tensor_tensor(
                        out=den1[:], in0=den[:],
                        in1=npad_sb[:, t:t + 1].broadcast_to([P, H]),
                        op=OP.subtract)
                    nc.vector.tensor_scalar_max(out=den1[:], in0=den1[:],
                                                scalar1=1e-16)
                    rcp = sp.tile([P, H], f32, tag="rc")
                    nc.vector.reciprocal(out=rcp[:], in_=den1[:])
                    outm = pp2.tile([P, HID], f32, tag="om")
                    nc.vector.tensor_tensor(
                        out=outm[:].rearrange("p (h d) -> p h d", h=H),
                        in0=num[:].rearrange("p (h d) -> p h d", h=H),
                        in1=rcp[:].unsqueeze(2).broadcast_to([P, H, DH]),
                        op=OP.mult)
                    gl = pp2.tile([P, HID], f32, tag="gl")
                    nc.scalar.activation(out=gl[:], in_=outm[:], func=ACT.Gelu)
                    pst = pT.tile([P, P], f32, tag="t1")
                    nc.tensor.transpose(pst[:], gl[:], sb["ident"][:])
                    tg = pp2.tile([P, HID], f32, tag="tg")
                    nc.vector.tensor_copy(out=tg[:], in_=pst[:])
                    ps2 = pM.tile([P, HID], f32, tag="m")
                    nc.tensor.matmul(out=ps2[:], lhsT=tg[:], rhs=wout[:],
                                     start=True, stop=False)
                    nc.tensor.matmul(out=ps2[:], lhsT=ones_m, rhs=bout[:],
                                     start=False, stop=False)
                    nc.tensor.matmul(out=ps2[:], lhsT=xres[:, t * P:(t + 1) * P],
                                     rhs=omgi[:], start=False, stop=True)
                    xn = pp2.tile([P, HID], f32, tag="xn")
                    nc.scalar.activation(out=xn[:], in_=ps2[:], func=ACT.Relu)
                    pst2 = pT.tile([P, P], f32, tag="t2")
                    nc.tensor.transpose(pst2[:], xn[:], sb["ident"][:])
                    nc.vector.tensor_copy(out=xres[:, t * P:(t + 1) * P],
                                          in_=pst2[:])

                if phase_lim < 4:
                    continue
                with (
                    tc.tile_pool(name="gp", bufs=2) as gp,
                    tc.tile_pool(name="tp", bufs=2) as tp,
                    tc.tile_pool(name="sp", bufs=2) as sp,
                    tc.tile_pool(name="acc", bufs=2) as acc,
                    tc.tile_pool(name="pp2", bufs=2) as pp2,
                    tc.tile_pool(name="pT", bufs=2, space="PSUM") as pT,
                    tc.tile_pool(name="pM", bufs=2, space="PSUM") as pM,
                ):
                    # user tiles: groups iuA (item tab A), iuB (item tab B),
                    # uu (user tab cols 256:512)
                    itabA = itab[0:cfg.bsplit, 0:256]
                    for t in range(UT):
                        gts = []
                        if cfg.DA[t]:
                            gts.append((itabA, 256, cfg.baseA[t], int(cfg.DA[t])))
                        if cfg.DB[t]:
                            itabB = itab[cfg.bsplit:C * ipad, 0:256]
                            gts.append((itabB, 256, cfg.baseB[t], int(cfg.DB[t])))
                        if cfg.DU[t]:
                            gts.append((utab[:, 256:512], 512, cfg.baseU[t],
                                        int(cfg.DU[t])))
                        edge_tile(t, gts, qu, sb["npadu"], xu, sb[f"Woutu{l}"],
                                  sb[f"boutu{l}"], sb[f"OmgIu{l}"], ones_u(t),
                                  sb["idxu"])
                    for t in range(IT):
                        gts = []
                        if cfg.DI[t]:
                            gts.append((utab[:, 0:256], 512, cfg.baseI[t],
                                        int(cfg.DI[t])))
                        edge_tile(t, gts, qi, sb["npadi"], xi, sb[f"Wouti{l}"],
                                  sb[f"bouti{l}"], sb[f"OmgIi{l}"], ones_i(t),
                                  sb["idxi"])

            # ---- final shared linear ----
            with (
                tc.tile_pool(name="fo", bufs=3) as fo,
                tc.tile_pool(name="fp", bufs=2, space="PSUM") as fp,
            ):
                for (xres, nt, outd) in ((xu, UT, outu), (xi, IT, outi)):
                    for t in range(nt):
                        ps = fp.tile([P, 64], f32, tag="f")
                        nc.tensor.matmul(out=ps[:], lhsT=xres[:, t * P:(t + 1) * P],
                                         rhs=sb["Wlin"][:], start=True, stop=False)
                        nc.tensor.matmul(out=ps[:], lhsT=sb["ones"][:, 0:P],
                                         rhs=sb["blin"][:], start=False, stop=True)
                        of = fo.tile([P, 64], f32, tag="of")
                        nc.vector.tensor_copy(out=of[:], in_=ps[:])
                        nc.sync.dma_start(out=outd[t * P:(t + 1) * P, :], in_=of[:])

    nc.compile()
    return nc


# ---------------------------------------------------------------------------
# host driver
# ---------------------------------------------------------------------------

_HW_NS_TOTAL = 0


def _run(cfg, inp, prep, W):
    import time

    from concourse import bass_utils
    global _HW_NS_TOTAL

    nin_u = inp["x_user"].shape[1]
    t0 = time.time()
    nc = _build_program(cfg, nin_u)
    t1 = time.time()
    print(f"[kernel] build+compile: {t1 - t0:.1f}s", file=sys.stderr)

    # per-core inputs
    xu_full = np.asarray(inp["x_user"], np.float32)
    xi_full = np.asarray(inp["x_item"], np.float32)
    u_core, u_pos = prep["u_core"], prep["u_pos"]
    i_core, i_pos = prep["i_core"], prep["i_pos"]

    in_maps = []
    for c in range(cfg.C):
        m = {}
        xuT = np.zeros((nin_u, cfg.upad), np.float32)
        sel = u_core == c
        xuT[:, u_pos[sel]] = xu_full[sel].T
        m["xuT"] = xuT
        xiT = np.zeros((64, cfg.ipad), np.float32)
        sel = i_core == c
        xiT[:, i_pos[sel]] = xi_full[sel].T
        m["xiT"] = xiT
        for nm in ("Winu", "binu", "Wini", "bini", "Wlin", "blin"):
            m[nm] = W[nm]
        for l in range(L):
            for nm in ("Wbigu", "bbigu", "Wbigi", "bbigi", "Woutu", "boutu",
                       "OmgIu", "Wouti", "bouti", "OmgIi"):
                m[f"{nm}{l}"] = W[(nm, l)]
        m["idxu"] = prep["idx_u"][c]
        m["idxi"] = prep["idx_i"][c]
        m["npadu"] = np.ascontiguousarray(prep["npad_u"][c])
        m["npadi"] = np.ascontiguousarray(prep["npad_i"][c])
        in_maps.append(m)

    t0 = time.time()
    res = bass_utils.run_bass_kernel_spmd(nc, in_maps, core_ids=list(range(cfg.C)))
    dt_ns = int((time.time() - t0) * 1e9)
    if res.exec_time_ns:
        dt_ns = int(res.exec_time_ns)
    _HW_NS_TOTAL += dt_ns
    print(f"[kernel] launch wall: {dt_ns / 1e9:.2f}s", file=sys.stderr)

    outu = np.stack([res.results[c]["outu"] for c in range(cfg.C)])  # [C,upad,64]
    outi = np.stack([res.results[c]["outi"] for c in range(cfg.C)])
    out_user = outu[u_core, u_pos]
    out_item = outi[i_core, i_pos]
    return np.concatenate([out_user, out_item], 0).astype(np.float32)


def kernel(**inp):
    cfg = Cfg(int(inp["x_user"].shape[0]), int(inp["x_item"].shape[0]), 8)
    e_ui = (np.asarray(inp["edge_src_ui"]), np.asarray(inp["edge_dst_ui"]))
    e_iu = (np.asarray(inp["edge_src_iu"]), np.asarray(inp["edge_dst_iu"]))
    e_uu = (np.asarray(inp["edge_src_uu"]), np.asarray(inp["edge_dst_uu"]))
    prep = _prep_graph(cfg, e_ui, e_iu, e_uu)
    W = _fold_weights(inp)
    return _run(cfg, inp, prep, W)
